# revision 6
# baseline (speedup 1.0000x reference)
"""Trainium2 Bass kernel for a local-window self-attention block (v7.1).

Sharding: B=2 x 4 windows of 1024 tokens = 8 independent shards, one per
NeuronCore. No collectives.

v7 redesign vs v6 baseline:
  - bf16 matmul operands everywhere (fp32 PSUM accumulation): FWL weight
    loads, half the DMA traffic, half the SBUF footprint.
  - Host-side prep: x is fed both natural (f32, with b_out folded in for
    the residual) and pre-transposed (bf16) -> no on-device transposes.
    Weights are pre-tiled into matmul-friendly DRAM layouts.
  - Everything stays SBUF-resident; no DRAM scratch roundtrip.
  - Schedule interleaves QK-projection per head with attention so ScalarE
    exp hides under PE matmul work.
  - 2-bank PSUM tiles: each evacuation (exp / bias-add / normalize) is a
    single wide [128,1024] op instead of two [128,512] ops.
  - Softmax denominators: ones-column matmuls, optionally col-packed
    two-at-a-time via tile_position column groups (KPACK=1).
  - max-subtraction skipped (|scores| < ~6 for this regime; softmax is
    shift-invariant, fp32 exp range is fine).
"""

import numpy as np

T = 1024  # tokens per shard (window)
D = 1024
H = 8
DH = 128
P = 128
NT = T // P  # 8 token tiles
ND = D // P  # 8 d tiles
NCORES = 8
SCALE = 1.0 / float(np.sqrt(DH))
EPS = 1e-5

_CACHE = {}
import os as _os
PACK_DENOM = _os.environ.get("KPACK", "1") == "1"


def _emit_body(nc, tc, pools, aps, affine=True):
    import concourse.mybir as mybir

    f32 = mybir.dt.float32
    bf16 = mybir.dt.bfloat16
    AF = mybir.ActivationFunctionType
    ALU = mybir.AluOpType

    (xT_bf, xr32, wqk_l, wv_bf, wo_bf, b_qkv, ln_gamma, ln_beta, y) = aps
    (singles, bcast, xt, qkt, vres, wext, wqkp, ep, ctxp, rbp, rrowp,
     x2p, resp, statp, mm, psd) = pools

    def wide(ps):
        return ps[:].rearrange("p a b -> p (a b)")

    # ---- input loads first (ramp: xT/wv interleaved so V-proj k-steps can
    # start as soon as each (xT[k], wv[k]) pair lands), then constants ----
    xTt, wv = [], []
    for k in range(ND):
        t = xt.tile([P, T], bf16, tag="xt", name="xt_t")
        nc.sync.dma_start(out=t[:], in_=xT_bf[k * P : (k + 1) * P, :])
        xTt.append(t)
        t = wext.tile([P, D], bf16, tag="wext", name="wv_t")
        nc.sync.dma_start(out=t[:], in_=wv_bf[k * P : (k + 1) * P, :])
        wv.append(t)

    bqk = singles.tile([P, 2 * ND], f32, tag="bqk")
    nc.sync.dma_start(out=bqk[:], in_=b_qkv[0 : 2 * D].rearrange("(m p) -> p m", p=P))
    ones_bf = singles.tile([P, 1], bf16, tag="ones")
    nc.vector.memset(ones_bf[:], 1.0)
    eps_t = singles.tile([P, 1], f32, tag="eps")
    nc.vector.memset(eps_t[:], EPS)
    # broadcast constants: DMA one row each, replicate on GpSimd (cheaper
    # than DMA-replicating 512KB per tensor)
    crow = singles.tile([1, 3 * D], f32, tag="crow")
    nc.sync.dma_start(out=crow[0:1, 0:D], in_=b_qkv[2 * D : 3 * D].rearrange("(o d) -> o d", o=1))
    nc.sync.dma_start(out=crow[0:1, D : 2 * D], in_=ln_gamma.rearrange("(o d) -> o d", o=1))
    nc.sync.dma_start(out=crow[0:1, 2 * D : 3 * D], in_=ln_beta.rearrange("(o d) -> o d", o=1))
    bv_bc = bcast.tile([P, D], f32, tag="bv")
    nc.gpsimd.partition_broadcast(bv_bc[:], crow[0:1, 0:D])
    gamma_bc = bcast.tile([P, D], f32, tag="gamma")
    nc.gpsimd.partition_broadcast(gamma_bc[:], crow[0:1, D : 2 * D])
    beta_bc = bcast.tile([P, D], f32, tag="beta")
    nc.gpsimd.partition_broadcast(beta_bc[:], crow[0:1, 2 * D : 3 * D])

    # ---- per-head attention, software-pipelined. Emission (=priority)
    # order per window h: scores(h) -> ctx(h-1) -> denom(h) -> QK(h+1), so
    # PE always has exp-independent filler work (prev ctx, next QK proj)
    # while ScalarE drains the softmax exps. The V projection is emitted
    # right after scores(0) so it fills head 0's exp window. ----
    qkT = [None] * (2 * ND)
    Eh = [None] * H
    rbh = [None] * H
    ctxT = [None] * H
    V = []
    wo = []

    def emit_qk(h):
        for m in (h, ND + h):
            wq = wqkp.tile([P, ND, P], bf16, tag="wqk", name="wqk_t")
            nc.sync.dma_start(
                out=wq[:], in_=wqk_l[:, m * (ND * P) : (m + 1) * (ND * P)]
            )
            ps = mm.tile([P, 2, 512], f32, tag="mm", name="mm_t")
            for k in range(ND):
                for ch in range(2):
                    nc.tensor.matmul(
                        ps[:, ch, :],
                        wq[:, k, :],
                        xTt[k][:, ch * 512 : (ch + 1) * 512],
                        start=(k == 0),
                        stop=(k == ND - 1),
                    )
            qt = qkt.tile([P, T], bf16, tag="qkt", name="qkt_t")
            nc.vector.tensor_scalar_add(qt[:], wide(ps), bqk[:, m : m + 1])
            qkT[m] = qt

    def emit_scores(h):
        qT, kT = qkT[h], qkT[ND + h]
        E = ep.tile([P, NT, T], bf16, tag="e", name="e_t")
        Eh[h] = E
        for kt in range(NT):
            ps_s = mm.tile([P, 2, 512], f32, tag="mm", name="mm_t")
            for ch in range(2):
                nc.tensor.matmul(
                    ps_s[:, ch, :],
                    kT[:, kt * P : (kt + 1) * P],
                    qT[:, ch * 512 : (ch + 1) * 512],
                    start=True,
                    stop=True,
                )
            nc.scalar.activation(
                out=E[:, kt, :], in_=wide(ps_s), func=AF.Exp,
                bias=0.0, scale=SCALE,
            )

    def emit_ctx(h):
        psc = mm.tile([P, 2, 512], f32, tag="mm", name="mm_t")
        for kt in range(NT):
            for ch in range(2):
                nc.tensor.matmul(
                    psc[:, ch, :],
                    V[kt][:, h * P : (h + 1) * P],
                    Eh[h][:, kt, ch * 512 : (ch + 1) * 512],
                    start=(kt == 0),
                    stop=(kt == NT - 1),
                )
        ct = ctxp.tile([P, T], bf16, tag="ctx", name="ctx_t")
        nc.vector.tensor_mul(ct[:], wide(psc), rbh[h][:])
        ctxT[h] = ct

    def emit_vproj():
        for tt in range(NT):
            ps = mm.tile([P, 2, 512], f32, tag="mm", name="mm_t")
            for k in range(ND):
                for ch in range(2):
                    nc.tensor.matmul(
                        ps[:, ch, :],
                        xTt[k][:, tt * P : (tt + 1) * P],
                        wv[k][:, ch * 512 : (ch + 1) * 512],
                        start=(k == 0),
                        stop=(k == ND - 1),
                    )
            vt = vres.tile([P, D], bf16, tag="v", name="v_t")
            nc.vector.tensor_add(vt[:], wide(ps), bv_bc[:])
            V.append(vt)
        # out-proj weights prefetch (slots shared with wv, frees after V)
        for k in range(ND):
            t = wext.tile([P, D], bf16, tag="wext", name="wo_t")
            nc.sync.dma_start(out=t[:], in_=wo_bf[k * P : (k + 1) * P, :])
            wo.append(t)

    def emit_scores_with_ctx(h):
        # interleave scores(h) per key-tile with ctx(h-1): the ctx
        # accumulator takes one PSUM slot up front and its matmuls act as
        # exp-independent filler between exp-gated scores tiles
        qT, kT = qkT[h], qkT[ND + h]
        E = ep.tile([P, NT, T], bf16, tag="e", name="e_t")
        Eh[h] = E
        hp = h - 1
        psc = mm.tile([P, 2, 512], f32, tag="mm", name="mm_t")
        for kt in range(NT):
            ps_s = mm.tile([P, 2, 512], f32, tag="mm", name="mm_t")
            for ch in range(2):
                nc.tensor.matmul(
                    ps_s[:, ch, :],
                    kT[:, kt * P : (kt + 1) * P],
                    qT[:, ch * 512 : (ch + 1) * 512],
                    start=True,
                    stop=True,
                )
            nc.scalar.activation(
                out=E[:, kt, :], in_=wide(ps_s), func=AF.Exp,
                bias=0.0, scale=SCALE,
            )
            for ch in range(2):
                nc.tensor.matmul(
                    psc[:, ch, :],
                    V[kt][:, hp * P : (hp + 1) * P],
                    Eh[hp][:, kt, ch * 512 : (ch + 1) * 512],
                    start=(kt == 0),
                    stop=(kt == NT - 1),
                )
        ct = ctxp.tile([P, T], bf16, tag="ctx", name="ctx_t")
        nc.vector.tensor_mul(ct[:], wide(psc), rbh[hp][:])
        ctxT[hp] = ct

    emit_vproj()
    emit_qk(0)
    emit_qk(1)
    for h in range(H):
        if h == 0:
            emit_scores(h)
        else:
            emit_scores_with_ctx(h)

        # denominators: ones-column matmuls; optionally col-packed (col
        # groups 0 and 32 run concurrently on the PE array)
        r_row = rrowp.tile([33, T], f32, tag="rrow", name="rrow_t")
        if PACK_DENOM:
            psd0 = psd.tile([1, 512], f32, tag="psd", name="psd0_t")
            psd1 = psd.tile([33, 512], f32, tag="psd", name="psd1_t")
            for kt in range(NT):
                nc.tensor.matmul(
                    psd0[:], ones_bf[:], Eh[h][:, kt, 0:512],
                    start=(kt == 0), stop=(kt == NT - 1),
                )
                nc.tensor.matmul(
                    psd1[32:33, :], ones_bf[:], Eh[h][:, kt, 512:1024],
                    start=(kt == 0), stop=(kt == NT - 1),
                )
            nc.vector.reciprocal(r_row[0:1, 0:512], psd0[:])
            nc.vector.reciprocal(r_row[0:1, 512:1024], psd1[32:33, :])
        else:
            for ch in range(2):
                psd_t = psd.tile([1, 512], f32, tag="psd", name="psd0_t")
                for kt in range(NT):
                    nc.tensor.matmul(
                        psd_t[:], ones_bf[:], Eh[h][:, kt, ch * 512 : (ch + 1) * 512],
                        start=(kt == 0), stop=(kt == NT - 1),
                    )
                nc.vector.reciprocal(r_row[0:1, ch * 512 : (ch + 1) * 512], psd_t[:])
        rb = rbp.tile([P, T], f32, tag="rb", name="rb_t")
        nc.gpsimd.partition_broadcast(rb[:], r_row[0:1, :])
        rbh[h] = rb
        if h + 2 < H:
            emit_qk(h + 2)

    emit_ctx(H - 1)

    # ---- out projection + residual + LayerNorm ----
    for tt in range(NT):
        x2 = x2p.tile([P, D], f32, tag="x2", name="x2_t")
        nc.sync.dma_start(out=x2[:], in_=xr32[tt * P : (tt + 1) * P, :])
        pso = mm.tile([P, 2, 512], f32, tag="mm", name="mm_t")
        for k in range(ND):
            for ch in range(2):
                nc.tensor.matmul(
                    pso[:, ch, :],
                    ctxT[k][:, tt * P : (tt + 1) * P],
                    wo[k][:, ch * 512 : (ch + 1) * 512],
                    start=(k == 0),
                    stop=(k == ND - 1),
                )
        res = resp.tile([P, D], f32, tag="res", name="res_t")
        nc.vector.tensor_add(res[:], wide(pso), x2[:])

        # LayerNorm over D (free axis)
        stats = statp.tile([P, 2, 6], f32, tag="stats", name="stats_t")
        mv = statp.tile([P, 2], f32, tag="mv", name="mv_t")
        grouped = res[:].rearrange("p (g d) -> p g d", g=2)
        for g in range(2):
            nc.vector.bn_stats(out=stats[:, g, :], in_=grouped[:, g, :])
        nc.vector.bn_aggr(out=mv[:], in_=stats[:])
        # rstd = (var+eps)^-0.5 = exp(-0.5*ln(var+eps)); keeps every ACT
        # call in the natural_log_exp_and_others table set (no per-iter
        # table reloads; see _patch_act_tables)
        rstd = statp.tile([P, 1], f32, tag="rstd", name="rstd_t")
        nc.scalar.activation(
            out=rstd[:], in_=mv[:, 1:2], func=AF.Ln, bias=eps_t[:], scale=1.0
        )
        nc.scalar.activation(
            out=rstd[:], in_=rstd[:], func=AF.Exp, bias=0.0, scale=-0.5
        )
        # normalize on ScalarE (idle in this phase): res*rstd - mu*rstd
        nbias = statp.tile([P, 1], f32, tag="nbias", name="nbias_t")
        nc.vector.tensor_scalar(
            out=nbias[:],
            in0=mv[:, 0:1],
            scalar1=rstd[:],
            scalar2=-1.0,
            op0=ALU.mult,
            op1=ALU.mult,
        )
        normed = resp.tile([P, D], f32, tag="normed", name="normed_t")
        nc.scalar.activation(
            out=normed[:], in_=res[:], func=AF.Identity,
            bias=nbias[:], scale=rstd[:],
        )
        if affine:
            nc.vector.tensor_mul(normed[:], normed[:], gamma_bc[:])
            nc.gpsimd.tensor_add(normed[:], normed[:], beta_bc[:])
        nc.sync.dma_start(out=y[tt * P : (tt + 1) * P, :], in_=normed[:])


_ACT_PATCHED = False


def _patch_act_tables():
    """Steer the act-table-set chooser to `natural_log_exp_and_others` for
    Exp and Ln (the only transcendentals this kernel uses) by masking them
    out of the earlier single-function sets. The emitted act_func_set_id
    still indexes the canonical act_info.json order, and that set really
    contains both functions, so the hardware tables are valid — this just
    avoids reloading tables between softmax (Exp) and layernorm (Ln)."""
    global _ACT_PATCHED
    if _ACT_PATCHED:
        return
    _ACT_PATCHED = True
    import concourse.mybir as mybir
    import concourse.hw_specs as hw_specs
    import concourse.bacc as bacc_mod

    AF = mybir.ActivationFunctionType
    orig = hw_specs.get_activation_tables

    def patched(arch):
        d = orig(arch)
        out = {}
        for name, fns in d.items():
            fns = set(fns)
            if name != "natural_log_exp_and_others":
                fns.discard(AF.Exp)
                fns.discard(AF.Ln)
            out[name] = fns
        return out

    bacc_mod.get_activation_tables = patched


def build(n_iters: int = 1, stag: bool | None = None, affine: bool = True):
    import concourse.mybir as mybir
    import concourse.tile as tile
    from concourse import bacc

    _patch_act_tables()
    if stag is None:
        stag = _os.environ.get("KSTAG", "1") == "1"

    f32 = mybir.dt.float32
    bf16 = mybir.dt.bfloat16

    nc = bacc.Bacc("TRN2", target_bir_lowering=False, debug=False, num_devices=NCORES)
    xT_bf = nc.dram_tensor("xT_bf", [D, T], bf16, kind="ExternalInput").ap()
    xr32 = nc.dram_tensor("xr32", [T, D], f32, kind="ExternalInput").ap()
    wqk_l = nc.dram_tensor("wqk_l", [P, 2 * ND * ND * P], bf16, kind="ExternalInput").ap()
    wv_bf = nc.dram_tensor("wv_bf", [D, D], bf16, kind="ExternalInput").ap()
    wo_bf = nc.dram_tensor("wo_bf", [D, D], bf16, kind="ExternalInput").ap()
    b_qkv = nc.dram_tensor("b_qkv", [3 * D], f32, kind="ExternalInput").ap()
    ln_gamma = nc.dram_tensor("ln_gamma", [D], f32, kind="ExternalInput").ap()
    ln_beta = nc.dram_tensor("ln_beta", [D], f32, kind="ExternalInput").ap()
    y = nc.dram_tensor("y", [T, D], f32, kind="ExternalOutput").ap()
    aps = (xT_bf, xr32, wqk_l, wv_bf, wo_bf, b_qkv, ln_gamma, ln_beta, y)

    with tile.TileContext(nc) as tc:
        with (
            tc.tile_pool(name="singles", bufs=1) as singles,
            tc.tile_pool(name="bcast", bufs=1) as bcast,
            tc.tile_pool(name="xt", bufs=8) as xt,
            tc.tile_pool(name="qkt", bufs=16) as qkt,
            tc.tile_pool(name="vres", bufs=8) as vres,
            tc.tile_pool(name="wext", bufs=8) as wext,
            tc.tile_pool(name="wqkp", bufs=4) as wqkp,
            tc.tile_pool(name="ep", bufs=2) as ep,
            tc.tile_pool(name="ctxp", bufs=8) as ctxp,
            tc.tile_pool(name="rbp", bufs=2) as rbp,
            tc.tile_pool(name="rrowp", bufs=2) as rrowp,
            tc.tile_pool(name="x2p", bufs=3) as x2p,
            tc.tile_pool(name="resp", bufs=2) as resp,
            tc.tile_pool(name="statp", bufs=6) as statp,
            tc.tile_pool(name="mm", bufs=3, space="PSUM") as mm,
            tc.tile_pool(name="psd", bufs=2, space="PSUM") as psd,
        ):
            pools = (singles, bcast, xt, qkt, vres, wext, wqkp, ep, ctxp,
                     rbp, rrowp, x2p, resp, statp, mm, psd)
            if n_iters == 1:
                _emit_body(nc, tc, pools, aps, affine)
            else:
                with tc.For_i(
                    0, n_iters, 1,
                    hint_engines=(mybir.EngineType.PE,),
                    staggered_reset=stag,
                ):
                    _emit_body(nc, tc, pools, aps, affine)
    nc.compile()
    return nc


def _get_nc(n_iters: int = 1, stag: bool | None = None, affine: bool = True):
    if stag is None:
        stag = _os.environ.get("KSTAG", "1") == "1"
    key = (n_iters, stag, affine)
    if key not in _CACHE:
        _CACHE[key] = build(n_iters, stag, affine)
    return _CACHE[key]


def _shard_inputs(inputs):
    import ml_dtypes

    bf16 = ml_dtypes.bfloat16
    val = np.ascontiguousarray(inputs["val"], dtype=np.float32)
    B, S, Dm = val.shape
    shards = val.reshape(B * (S // T), T, Dm)

    w_qkv = np.ascontiguousarray(inputs["w_qkv"], dtype=np.float32)
    w_out = np.ascontiguousarray(inputs["w_out"], dtype=np.float32)
    b_qkv = np.ascontiguousarray(inputs["b_qkv"], dtype=np.float32)
    b_out = np.ascontiguousarray(inputs["b_out"], dtype=np.float32)

    # QK weight tiles: [p, m, k, n] with row r=k*128+p, col c=m*128+n
    wqk_l = np.ascontiguousarray(
        w_qkv[:, : 2 * D]
        .reshape(ND, P, 2 * ND, P)
        .transpose(1, 2, 0, 3)
        .reshape(P, 2 * ND * ND * P)
        .astype(bf16)
    )
    wv_bf = np.ascontiguousarray(w_qkv[:, 2 * D :].astype(bf16))
    wo_bf = np.ascontiguousarray(w_out.astype(bf16))

    shared = {
        "wqk_l": wqk_l,
        "wv_bf": wv_bf,
        "wo_bf": wo_bf,
        "b_qkv": b_qkv,
        "ln_gamma": np.ascontiguousarray(inputs["ln_gamma"], dtype=np.float32),
        "ln_beta": np.ascontiguousarray(inputs["ln_beta"], dtype=np.float32),
    }
    in_maps = []
    for i in range(NCORES):
        x = shards[i]
        m = {
            "xT_bf": np.ascontiguousarray(x.T.astype(bf16)),
            "xr32": np.ascontiguousarray(x + b_out[None, :]),
        }
        m.update(shared)
        in_maps.append(m)
    return in_maps, (B, S, Dm)


def _setup_jax_cache():
    import os

    d = os.environ.get("JAX_COMPILATION_CACHE_DIR") or os.path.expanduser(
        "~/.cache/bass_kernel_jax_cache"
    )
    try:
        os.makedirs(d, exist_ok=True)
        import jax

        jax.config.update("jax_compilation_cache_dir", d)
        jax.config.update("jax_persistent_cache_min_compile_time_secs", 1.0)
    except Exception:
        pass


def _needs_affine(inputs):
    g = np.asarray(inputs["ln_gamma"], dtype=np.float32)
    b = np.asarray(inputs["ln_beta"], dtype=np.float32)
    return not (np.all(g == 1.0) and np.all(b == 0.0))


def run_on_cores(inputs, n_iters: int = 1, stag: bool | None = None):
    _setup_jax_cache()
    from concourse.bass_utils import run_bass_kernel_spmd

    nc = _get_nc(n_iters, stag, _needs_affine(inputs))
    in_maps, shape = _shard_inputs(inputs)
    res = run_bass_kernel_spmd(nc, in_maps, list(range(NCORES)))
    B, S, Dm = shape
    out = np.stack([res.results[i]["y"] for i in range(NCORES)], axis=0)
    return out.reshape(B, S, Dm)


def kernel(**inputs) -> np.ndarray:
    return run_on_cores(inputs, n_iters=1)
